# revision 17
# baseline (speedup 1.0000x reference)
"""Trainium2 Bass kernel for Attention4DDownsample (EfficientFormer-style).

Strategy: pure data parallelism over batch (256 -> 32 examples/core x 8 cores).
All BN scales/biases + attention scale folded into conv weights on host.
fp16 on the PE; fp32 PSUM accumulation. x and v4 carry a zero halo (16x16
spatial) so depthwise taps are full-rect matmuls; the q-path depthwise conv is
folded into the q 1x1 projection, and the avg-pool branch is folded into the
center tap (9 fused weight sets).

v2: attention runs on example PAIRS via block-diagonal stationaries:
S for (head, pair) = [q_e0|0|I ; 0|q_e1|I]^T @ [k_e0 ; k_e1 ; bias] in one
matmul -> PSUM rows 0:49 = S(e0), 49:98 = S(e1). Two heads share one
[128,512] PSUM bank so exp runs 512 wide. Softmax: exp (Act) -> per-row sums
(DVE reduce) -> reciprocal -> scale (Pool). P^T via PE transposes whose
identity is column-sliced to 98 to drop garbage rows. PSUM evictions are
balanced across Act/DVE (gpsimd has no PSUM port); v_local runs as
diag-weight matmuls accumulated in PSUM; attention output accumulates on
top, then fused ReLU+bias, projection matmuls, and the store.
"""

import sys

sys.path.insert(0, "/opt/trn_rl_repo")

import numpy as np

B, C, H, W = 256, 384, 14, 14
HEADS, KD, D = 8, 16, 64
NHKD, DH = 128, 512
OUT = 384
N2, N = 49, 196
HP, WP = 16, 16  # padded spatial
NP = HP * WP  # 256
NCORES = 8
EPC = B // NCORES  # 32 examples per core
SG = 8  # examples per super-group
NPAIR = SG // 2  # 4 example pairs per super-group

TAPS = [(dy, dx) for dy in range(3) for dx in range(3)]


def build_bass(epc=EPC, reps=1):
    import concourse.bass as bass
    import concourse.tile as tile
    from concourse import bacc, mybir

    f16 = mybir.dt.float16
    f32 = mybir.dt.float32
    AF = mybir.ActivationFunctionType

    nsg = epc // SG
    nc = bacc.Bacc(trn_type="TRN2", debug=False)

    # ---- DRAM I/O ----
    x_t = nc.dram_tensor("x", (3, 128, epc, NP), f16, kind="ExternalInput")
    qw_t = nc.dram_tensor("qw", (128, 3, 9, 128), f16, kind="ExternalInput")
    kw_t = nc.dram_tensor("kw", (128, 3, 128), f16, kind="ExternalInput")
    vw_t = nc.dram_tensor("vw", (128, 3, 4, 128), f16, kind="ExternalInput")
    pw_t = nc.dram_tensor("pw", (128, 4, 3, 128), f16, kind="ExternalInput")
    vld_t = nc.dram_tensor("vld", (128, 4, 9, 128), f16, kind="ExternalInput")
    bias_t = nc.dram_tensor("bias", (49, 8, 256), f16, kind="ExternalInput")
    iden_t = nc.dram_tensor("iden", (128, 128), f16, kind="ExternalInput")
    # bvec cols: vb[0:4] pb[4:7] vlb[7:11] kb[11] qb[12]
    bvec_t = nc.dram_tensor("bvec", (128, 13), f32, kind="ExternalInput")
    y_t = nc.dram_tensor("y", (3, 128, epc, 49), f32, kind="ExternalOutput")

    with tile.TileContext(nc) as tc:
        with (
            tc.tile_pool(name="consts", bufs=1) as consts,
            tc.tile_pool(name="xp", bufs=2) as xp,
            tc.tile_pool(name="sb_small", bufs=3) as sbs,
            tc.tile_pool(name="sb_kv", bufs=2) as sbkv,
            tc.tile_pool(name="sb_vt", bufs=10) as sbvt,
            tc.tile_pool(name="sb_a", bufs=6) as sba,
            tc.tile_pool(name="sb_at", bufs=16) as sbat,
            tc.tile_pool(name="sb_z", bufs=3) as sbz,
            tc.tile_pool(name="sb_y", bufs=4) as sby,
            tc.tile_pool(name="psA", bufs=1, space="PSUM") as psA,
            tc.tile_pool(name="psS", bufs=1, space="PSUM") as psS,
            tc.tile_pool(name="psT", bufs=1, space="PSUM") as psT,
            tc.tile_pool(name="psVL", bufs=1, space="PSUM") as psVL,
        ):
            # ---- load constants ----
            qw_sb = consts.tile([128, 3 * 9 * 128], f16, name="qw_sb")
            nc.sync.dma_start(out=qw_sb, in_=qw_t.ap().rearrange("p a b c -> p (a b c)"))
            kw_sb = consts.tile([128, 3 * 128], f16, name="kw_sb")
            nc.sync.dma_start(out=kw_sb, in_=kw_t.ap().rearrange("p a b -> p (a b)"))
            vw_sb = consts.tile([128, 3 * 4 * 128], f16, name="vw_sb")
            nc.sync.dma_start(out=vw_sb, in_=vw_t.ap().rearrange("p a b c -> p (a b c)"))
            pw_sb = consts.tile([128, 4 * 3 * 128], f16, name="pw_sb")
            nc.sync.dma_start(out=pw_sb, in_=pw_t.ap().rearrange("p a b c -> p (a b c)"))
            vld_sb = consts.tile([128, 4 * 9 * 128], f16, name="vld_sb")
            nc.sync.dma_start(out=vld_sb, in_=vld_t.ap().rearrange("p a b c -> p (a b c)"))
            iden_sb = consts.tile([128, 128], f16, name="iden_sb")
            nc.sync.dma_start(out=iden_sb, in_=iden_t.ap())
            bvec_sb = consts.tile([128, 13], f32, name="bvec_sb")
            nc.sync.dma_start(out=bvec_sb, in_=bvec_t.ap())

            qw_l = lambda kc, t: qw_sb[:, (kc * 9 + t) * 128:][:, :128]
            kw_l = lambda kc: kw_sb[:, kc * 128:][:, :128]
            vw_l = lambda kc, m: vw_sb[:, (kc * 4 + m) * 128:][:, :128]
            pw_l = lambda r, m: pw_sb[:, (r * 3 + m) * 128:][:, :128]
            vld_l = lambda r, t: vld_sb[:, (r * 9 + t) * 128:][:, :128]

            # ---- persistent augmented per-head q/k tiles (ping-pong) ----
            # rows 0:16 = per-sg head data (one contiguous DMA), rows 16:65 =
            # const identity (q) / bias table (k): S = q@k + I@bias fused.
            q2 = [[consts.tile([65, SG * 49], f16, name=f"q2_{b}_{h}")
                   for h in range(8)] for b in range(2)]
            k2 = [consts.tile([65, SG * 256], f16, name=f"k2_{h}")
                  for h in range(8)]
            it = iden_t.ap()
            bt = bias_t.ap()
            for b in range(2):
                for h in range(8):
                    nc.sync.dma_start(
                        out=q2[b][h][16:65, :],
                        in_=bass.AP(tensor=it.tensor, offset=0,
                                    ap=[[128, 49], [0, SG], [1, 49]]),
                    )
            for h in range(8):
                nc.sync.dma_start(
                    out=k2[h][16:65, :],
                    in_=bass.AP(tensor=bt.tensor, offset=h * 256,
                                ap=[[8 * 256, 49], [0, SG], [1, 256]]),
                )

            from contextlib import nullcontext

            loop_cm = tc.For_i(0, reps, 1) if reps > 1 else nullcontext()
            with loop_cm:
              for sg in range(nsg):
                e0 = sg * SG
                pp = sg % 2  # ping-pong index for q2/k2
                # ---- x for this super-group (host-padded 16x16) ----
                x_all = xp.tile([128, 3 * SG * NP], f16, name="x_sb", tag="x")
                nc.sync.dma_start(
                    out=x_all,
                    in_=bass.AP(
                        tensor=x_t.ap().tensor, offset=e0 * NP,
                        ap=[[epc * NP, 128], [128 * epc * NP, 3],
                            [NP, SG], [1, NP]],
                    ),
                )
                xr = [
                    x_all[:, kc * SG * NP : (kc + 1) * SG * NP].rearrange(
                        "p (e h w) -> p e h w", e=SG, h=HP
                    )
                    for kc in range(3)
                ]

                # ================= Q path (8 examples) =================
                ps_q = psA.tile([128, SG * 49], f32, name="ps_q", tag="ps_qk", bufs=1)
                for t in range(9):
                    dy, dx = TAPS[t]
                    for kc in range(3):
                        nc.tensor.matmul(
                            ps_q, qw_l(kc, t),
                            xr[kc][:, :, dy : dy + 13 : 2, dx : dx + 13 : 2],
                            start=(t == 0 and kc == 0),
                            stop=(t == 8 and kc == 2),
                            skip_group_check=True,
                        )
                q_sb = sbs.tile([128, SG * 49], f16, name="q_sb", tag="q")
                nc.scalar.activation(
                    out=q_sb, in_=ps_q, func=AF.Identity,
                    bias=bvec_sb[:, 12:13], scale=1.0,
                )

                # ================= K / V convs (pairs of 2 examples) ===
                k_sb = sbkv.tile([128, SG * NP], f16, name="k_sb", tag="k")
                kp = k_sb.rearrange("p (e h w) -> p e h w", e=SG, h=HP)
                if sg < 2:
                    nc.gpsimd.memset(kp[:, :, 0:16:15, :], 0.0)
                    nc.gpsimd.memset(kp[:, :, 1:15, 0:16:15], 0.0)
                v4_sb = [
                    sbkv.tile([128, SG * NP], f16, name="v4_sb", tag=f"v4_{r}")
                    for r in range(4)
                ]
                for r in range(4):
                    if sg < 2:
                        v4p = v4_sb[r].rearrange("p (e h w) -> p e h w", e=SG, h=HP)
                        nc.gpsimd.memset(v4p[:, :, 0:16:15, :], 0.0)
                        nc.gpsimd.memset(v4p[:, :, 1:15, 0:16:15], 0.0)

                for p2 in range(NPAIR):
                    es = p2 * 2
                    xin = [xr[kc][:, es : es + 2, 1:15, 1:15] for kc in range(3)]
                    ps_k = psA.tile([128, 2 * N], f32, name="ps_k", tag="ps_qk", bufs=1)
                    for kc in range(3):
                        nc.tensor.matmul(
                            ps_k, kw_l(kc), xin[kc],
                            start=(kc == 0), stop=(kc == 2),
                        )
                    nc.scalar.activation(
                        out=kp[:, es : es + 2, 1:15, 1:15], in_=ps_k,
                        func=AF.Identity, bias=bvec_sb[:, 11:12], scale=1.0,
                    )
                    for m in range(4):
                        ps_v = psA.tile([128, 2 * N], f32, name="ps_v", tag="ps_v", bufs=2)
                        for kc in range(3):
                            nc.tensor.matmul(
                                ps_v, vw_l(kc, m), xin[kc],
                                start=(kc == 0), stop=(kc == 2),
                            )
                        v4pm = v4_sb[m].rearrange("p (e h w) -> p e h w", e=SG, h=HP)
                        if m < 2:
                            nc.scalar.activation(
                                out=v4pm[:, es : es + 2, 1:15, 1:15], in_=ps_v,
                                func=AF.Identity, bias=bvec_sb[:, m : m + 1], scale=1.0,
                            )
                        else:
                            nc.vector.tensor_scalar_add(
                                out=v4pm[:, es : es + 2, 1:15, 1:15], in0=ps_v,
                                scalar1=bvec_sb[:, m : m + 1],
                            )

                # ---- per-head data rows into block-diag tiles (DMA) ----
                for h in range(8):
                    nc.sync.dma_start(
                        out=q2[pp][h][0:16, :],
                        in_=q_sb[16 * h : 16 * h + 16, :],
                    )
                    nc.sync.dma_start(
                        out=k2[h][0:16, :],
                        in_=k_sb[16 * h : 16 * h + 16, :],
                    )

                # ======== v4 -> v4T transposes (padded-flat halves) ====
                # half 0 = padded rows 0..7 (flat n' 0:128), half 1 = 8..15
                v4t = [[], []]
                for ee in range(SG):
                    ta = sbvt.tile([128, 512], f16, name="v4t_a", tag="v4t_a")
                    tb = sbvt.tile([128, 512], f16, name="v4t_b", tag="v4t_b")
                    for m in range(4):
                        for half, dst in ((0, ta), (1, tb)):
                            ps_t = psT.tile([128, 128], f16, name="ps_t", tag="pst", bufs=2)
                            nc.tensor.transpose(
                                ps_t,
                                v4_sb[m][:, ee * NP + 128 * half : ee * NP + 128 * half + 128],
                                iden_sb,
                            )
                            if (m * 2 + half) in (2, 5):
                                nc.scalar.copy(
                                    out=dst[:, m * 128 : (m + 1) * 128], in_=ps_t
                                )
                            else:
                                nc.vector.tensor_copy(
                                    out=dst[:, m * 128 : (m + 1) * 128], in_=ps_t
                                )
                    v4t[0].append(ta)
                    v4t[1].append(tb)

                # ========= attention: S -> P -> P^T (per head2, pair) ==
                den = sbs.tile([128, 32], f32, name="den", tag="den")
                rden = sbs.tile([128, 32], f32, name="rden", tag="rden")
                at = {}
                for p in range(NPAIR):  # pair of examples 2p, 2p+1
                    a_pr = []
                    for h2 in range(4):
                        ps_s = psS.tile([128, 512], f32, name="ps_s", tag="ps_s", bufs=2)
                        for eo in range(2):
                            ee = 2 * p + eo
                            for hh in range(2):
                                h = 2 * h2 + hh
                                nc.tensor.matmul(
                                    ps_s[64 * hh : 64 * hh + 49,
                                         256 * eo : 256 * eo + 256],
                                    q2[pp][h][:, 49 * ee : 49 * ee + 49],
                                    k2[h][:, 256 * ee : 256 * ee + 256],
                                    start=True, stop=True,
                                )
                        ap = sba.tile([128, 512], f16, name="a_pr", tag="apr")
                        nc.scalar.activation(
                            out=ap[0:113, :], in_=ps_s[0:113, :], func=AF.Exp,
                        )
                        c = p * 8 + 2 * h2
                        nc.vector.reduce_sum(
                            out=den[0:113, c : c + 2],
                            in_=ap[0:113, :].rearrange("p (g n) -> p g n", g=2),
                            axis=mybir.AxisListType.X,
                        )
                        a_pr.append(ap)
                    nc.vector.reciprocal(
                        out=rden[0:113, 8 * p : 8 * p + 8],
                        in_=den[0:113, 8 * p : 8 * p + 8],
                    )
                    for h2 in range(4):
                        ap = a_pr[h2]
                        c = p * 8 + 2 * h2
                        for eo in range(2):
                            nc.vector.tensor_scalar_mul(
                                out=ap[0:113, 256 * eo : 256 * eo + 256],
                                in0=ap[0:113, 256 * eo : 256 * eo + 256],
                                scalar1=rden[0:113, c + eo : c + eo + 1],
                            )
                        att = sbat.tile([128, 4 * 113], f16, name="at", tag="at")
                        for bq in range(4):  # bq = 2*eo + half
                            ps_at = psT.tile([128, 128], f16, name="ps_at", tag="pst", bufs=2)
                            nc.tensor.transpose(
                                ps_at[:, 0:113],
                                ap[:, 128 * bq : 128 * bq + 128],
                                iden_sb[:, 0:113],
                            )
                            if bq == 3:
                                nc.scalar.copy(
                                    out=att[:, 113 * bq : 113 * bq + 113],
                                    in_=ps_at[:, 0:113],
                                )
                            else:
                                nc.vector.tensor_copy(
                                    out=att[:, 113 * bq : 113 * bq + 113],
                                    in_=ps_at[:, 0:113],
                                )
                        at[(h2, p)] = att

                # ==== v_local (diag matmuls) + AV accumulate + relu ====
                z_sb = []
                for r in range(4):
                    ps_vl = psVL.tile([128, SG * 49], f32, name="ps_vl", tag="ps_vl", bufs=1)
                    v4r = v4_sb[r].rearrange("p (e h w) -> p e h w", e=SG, h=HP)
                    for ti, (dy, dx) in enumerate(TAPS):
                        nc.tensor.matmul(
                            ps_vl,
                            vld_l(r, ti),
                            v4r[:, :, dy : dy + 13 : 2, dx : dx + 13 : 2],
                            start=(ti == 0), stop=False, skip_group_check=True,
                        )
                    for ee in range(SG):
                        p, par = ee // 2, ee % 2
                        att = at[(r, p)]
                        for hh in range(2):
                            h = 2 * r + hh
                            out_sl = ps_vl[
                                64 * hh : 64 * hh + 64, ee * 49 : (ee + 1) * 49
                            ]
                            for half in range(2):
                                co = 113 * (2 * par + half) + 64 * hh
                                nc.tensor.matmul(
                                    out_sl,
                                    v4t[half][ee][:, 64 * h : 64 * h + 64],
                                    att[:, co : co + 49],
                                    start=False,
                                    stop=(ee == SG - 1 and hh == 1 and half == 1),
                                    skip_group_check=True,
                                )
                    zz = sbz.tile([128, SG * 49], f16, name="z_sb", tag=f"z{r}")
                    nc.scalar.activation(
                        out=zz, in_=ps_vl, func=AF.Relu,
                        bias=bvec_sb[:, 7 + r : 8 + r], scale=1.0,
                    )
                    z_sb.append(zz)

                # ================= projection + store =================
                yy = sby.tile([128, 3 * SG * 49], f32, name="y_sb", tag="y")
                for m in range(3):
                    ps_y = psVL.tile([128, SG * 49], f32, name="ps_y", tag="ps_vl", bufs=1)
                    for r in range(4):
                        nc.tensor.matmul(
                            ps_y, pw_l(r, m), z_sb[r],
                            start=(r == 0), stop=(r == 3),
                        )
                    nc.scalar.activation(
                        out=yy[:, m * SG * 49 : (m + 1) * SG * 49], in_=ps_y,
                        func=AF.Identity,
                        bias=bvec_sb[:, 4 + m : 5 + m], scale=1.0,
                    )
                nc.sync.dma_start(
                    out=bass.AP(
                        tensor=y_t.ap().tensor, offset=e0 * 49,
                        ap=[[epc * 49, 128], [128 * epc * 49, 3],
                            [49, SG], [1, 49]],
                    ),
                    in_=yy,
                )
    nc.compile()
    return nc


def prep_weights(inputs):
    """Host-side: fold BN/scales, build device weight layouts."""
    f = lambda a: np.asarray(a, np.float32)
    scale = KD ** -0.5

    kw2 = f(inputs["k_w"])[:, :, 0, 0] * f(inputs["k_bn_s"])[:, None]  # (128,384)
    kb2 = f(inputs["k_b"]) * f(inputs["k_bn_s"]) + f(inputs["k_bn_b"])
    qw2 = f(inputs["q_proj_w"])[:, :, 0, 0] * f(inputs["q_bn_s"])[:, None] * scale
    qb2 = (
        f(inputs["q_proj_b"]) * f(inputs["q_bn_s"]) + f(inputs["q_bn_b"])
    ) * scale + qw2 @ f(inputs["q_local_b"])
    qlw = f(inputs["q_local_w"])[:, 0].reshape(C, 9)
    vw2 = f(inputs["v_w"])[:, :, 0, 0] * f(inputs["v_bn_s"])[:, None]  # (512,384)
    vb2 = f(inputs["v_b"]) * f(inputs["v_bn_s"]) + f(inputs["v_bn_b"])
    vlw = f(inputs["vl_w"])[:, 0].reshape(DH, 9) * f(inputs["vl_bn_s"])[:, None]
    vlb = f(inputs["vl_b"]) * f(inputs["vl_bn_s"]) + f(inputs["vl_bn_b"])
    pw2 = f(inputs["p_w"])[:, :, 0, 0] * f(inputs["p_bn_s"])[:, None]  # (384,512)
    pb2 = f(inputs["p_b"]) * f(inputs["p_bn_s"]) + f(inputs["p_bn_b"])

    # 9 taps; avg-pool branch folded into the center tap (dy,dx)=(1,1)=idx 4
    qw_arr = np.zeros((128, 3, 9, 128), np.float32)
    for kc in range(3):
        cs = slice(128 * kc, 128 * kc + 128)
        for t in range(9):
            lw = qlw[cs, t] + (1.0 if t == 4 else 0.0)
            qw_arr[:, kc, t, :] = qw2[:, cs].T * lw[:, None]
    kw_arr = np.zeros((128, 3, 128), np.float32)
    for kc in range(3):
        kw_arr[:, kc, :] = kw2[:, 128 * kc : 128 * kc + 128].T
    vw_arr = np.zeros((128, 3, 4, 128), np.float32)
    for kc in range(3):
        for m in range(4):
            vw_arr[:, kc, m, :] = vw2[128 * m : 128 * m + 128, 128 * kc : 128 * kc + 128].T
    pw_arr = np.zeros((128, 4, 3, 128), np.float32)
    for r in range(4):
        for m in range(3):
            pw_arr[:, r, m, :] = pw2[128 * m : 128 * m + 128, 128 * r : 128 * r + 128].T
    vld_arr = np.zeros((128, 4, 9, 128), np.float32)
    ii = np.arange(128)
    for r in range(4):
        for t in range(9):
            vld_arr[ii, r, t, ii] = vlw[128 * r : 128 * r + 128, t]

    # padded bias: (49, 8, 256) in 16x16 flat index; pad cols get -60000
    bt = f(inputs["bias_tab"]).transpose(1, 0, 2).reshape(49, 8, 14, 14)
    bias_arr = np.full((49, 8, HP, WP), -60000.0, np.float32)
    bias_arr[:, :, 1:15, 1:15] = bt
    bias_arr = bias_arr.reshape(49, 8, 256)

    bvec = np.zeros((128, 13), np.float32)
    for m in range(4):
        bvec[:, m] = vb2[128 * m : 128 * m + 128]
    for m in range(3):
        bvec[:, 4 + m] = pb2[128 * m : 128 * m + 128]
    for r in range(4):
        bvec[:, 7 + r] = vlb[128 * r : 128 * r + 128]
    bvec[:, 11] = kb2
    bvec[:, 12] = qb2

    return {
        "qw": qw_arr.astype(np.float16),
        "kw": kw_arr.astype(np.float16),
        "vw": vw_arr.astype(np.float16),
        "pw": pw_arr.astype(np.float16),
        "vld": vld_arr.astype(np.float16),
        "bias": bias_arr.astype(np.float16),
        "iden": np.eye(128, dtype=np.float16),
        "bvec": bvec,
    }


def prep_x_core(x, c, epc=EPC):
    """x (B, C, H, W) -> per-core (3, 128, epc, 256) fp16 with zero halo."""
    xc = np.asarray(x, np.float32)[c * epc : (c + 1) * epc]  # (epc, C, 14, 14)
    xp = np.zeros((epc, C, HP, WP), np.float32)
    xp[:, :, 1:15, 1:15] = xc
    xp = xp.reshape(epc, C, NP).transpose(1, 0, 2).reshape(3, 128, epc, NP)
    return xp.astype(np.float16)


def unpack_y(y, epc=EPC):
    """(3, 128, epc, 49) fp32 -> (epc, 384, 7, 7)."""
    return (
        np.asarray(y, np.float32)
        .reshape(OUT, epc, 49)
        .transpose(1, 0, 2)
        .reshape(epc, OUT, 7, 7)
    )


_CACHE = {}


def kernel(**inputs) -> np.ndarray:
    from concourse import bass_utils

    if "nc" not in _CACHE:
        _CACHE["nc"] = build_bass()
    nc = _CACHE["nc"]

    wmaps = prep_weights(inputs)
    in_maps = []
    for c in range(NCORES):
        m = dict(wmaps)
        m["x"] = prep_x_core(inputs["x"], c)
        in_maps.append(m)

    res = bass_utils.run_bass_kernel_spmd(nc, in_maps, core_ids=list(range(NCORES)))
    outs = [unpack_y(r["y"]) for r in res.results]
    return np.concatenate(outs, axis=0)


if __name__ == "__main__":
    print("building bass program...")
    nc = build_bass()
    print("build OK")


# revision 18
# speedup vs baseline: 1.5956x; 1.5956x over previous
"""Trainium2 Bass kernel for Attention4DDownsample (EfficientFormer-style).

Strategy: pure data parallelism over batch (256 -> 32 examples/core x 8 cores).
All BN scales/biases + attention scale folded into conv weights on host.
fp16 on the PE; fp32 PSUM accumulation. x and v4 carry a zero halo (16x16
spatial) so depthwise taps are full-rect matmuls; the q-path depthwise conv is
folded into the q 1x1 projection, and the avg-pool branch is folded into the
center tap (9 fused weight sets).

v2: attention runs on example PAIRS via block-diagonal stationaries:
S for (head, pair) = [q_e0|0|I ; 0|q_e1|I]^T @ [k_e0 ; k_e1 ; bias] in one
matmul -> PSUM rows 0:49 = S(e0), 49:98 = S(e1). Two heads share one
[128,512] PSUM bank so exp runs 512 wide. Softmax: exp (Act) -> per-row sums
(DVE reduce) -> reciprocal -> scale (Pool). P^T via PE transposes whose
identity is column-sliced to 98 to drop garbage rows. PSUM evictions are
balanced across Act/DVE (gpsimd has no PSUM port); v_local runs as
diag-weight matmuls accumulated in PSUM; attention output accumulates on
top, then fused ReLU+bias, projection matmuls, and the store.
"""

import sys

sys.path.insert(0, "/opt/trn_rl_repo")

import numpy as np

B, C, H, W = 256, 384, 14, 14
HEADS, KD, D = 8, 16, 64
NHKD, DH = 128, 512
OUT = 384
N2, N = 49, 196
HP, WP = 16, 16  # padded spatial
NP = HP * WP  # 256
NCORES = 8
EPC = B // NCORES  # 32 examples per core
SG = 8  # examples per super-group
NPAIR = SG // 2  # 4 example pairs per super-group

TAPS = [(dy, dx) for dy in range(3) for dx in range(3)]


def build_bass(epc=EPC, reps=1):
    import concourse.bass as bass
    import concourse.tile as tile
    from concourse import bacc, mybir

    f16 = mybir.dt.float16
    f32 = mybir.dt.float32
    AF = mybir.ActivationFunctionType

    nsg = epc // SG
    nc = bacc.Bacc(trn_type="TRN2", debug=False)

    # ---- DRAM I/O ----
    x_t = nc.dram_tensor("x", (3, 128, epc, NP), f16, kind="ExternalInput")
    qw_t = nc.dram_tensor("qw", (128, 3, 9, 128), f16, kind="ExternalInput")
    kw_t = nc.dram_tensor("kw", (128, 3, 128), f16, kind="ExternalInput")
    vw_t = nc.dram_tensor("vw", (128, 3, 4, 128), f16, kind="ExternalInput")
    pw_t = nc.dram_tensor("pw", (128, 4, 3, 128), f16, kind="ExternalInput")
    vld_t = nc.dram_tensor("vld", (128, 4, 9, 128), f16, kind="ExternalInput")
    bias_t = nc.dram_tensor("bias", (49, 8, 256), f16, kind="ExternalInput")
    iden_t = nc.dram_tensor("iden", (128, 128), f16, kind="ExternalInput")
    # bvec cols: vb[0:4] pb[4:7] vlb[7:11] kb[11] qb[12]
    bvec_t = nc.dram_tensor("bvec", (128, 13), f32, kind="ExternalInput")
    y_t = nc.dram_tensor("y", (3, 128, epc, 49), f32, kind="ExternalOutput")

    with tile.TileContext(nc) as tc:
        with (
            tc.tile_pool(name="consts", bufs=1) as consts,
            tc.tile_pool(name="xp", bufs=2) as xp,
            tc.tile_pool(name="sb_small", bufs=3) as sbs,
            tc.tile_pool(name="sb_kv", bufs=2) as sbkv,
            tc.tile_pool(name="sb_vt", bufs=10) as sbvt,
            tc.tile_pool(name="sb_a", bufs=6) as sba,
            tc.tile_pool(name="sb_at", bufs=16) as sbat,
            tc.tile_pool(name="sb_z", bufs=3) as sbz,
            tc.tile_pool(name="sb_y", bufs=4) as sby,
            tc.tile_pool(name="psA", bufs=1, space="PSUM") as psA,
            tc.tile_pool(name="psS", bufs=1, space="PSUM") as psS,
            tc.tile_pool(name="psT", bufs=1, space="PSUM") as psT,
            tc.tile_pool(name="psVL", bufs=1, space="PSUM") as psVL,
        ):
            # ---- load constants ----
            qw_sb = consts.tile([128, 3 * 9 * 128], f16, name="qw_sb")
            nc.sync.dma_start(out=qw_sb, in_=qw_t.ap().rearrange("p a b c -> p (a b c)"))
            kw_sb = consts.tile([128, 3 * 128], f16, name="kw_sb")
            nc.sync.dma_start(out=kw_sb, in_=kw_t.ap().rearrange("p a b -> p (a b)"))
            vw_sb = consts.tile([128, 3 * 4 * 128], f16, name="vw_sb")
            nc.sync.dma_start(out=vw_sb, in_=vw_t.ap().rearrange("p a b c -> p (a b c)"))
            pw_sb = consts.tile([128, 4 * 3 * 128], f16, name="pw_sb")
            nc.sync.dma_start(out=pw_sb, in_=pw_t.ap().rearrange("p a b c -> p (a b c)"))
            vld_sb = consts.tile([128, 4 * 9 * 128], f16, name="vld_sb")
            nc.sync.dma_start(out=vld_sb, in_=vld_t.ap().rearrange("p a b c -> p (a b c)"))
            iden_sb = consts.tile([128, 128], f16, name="iden_sb")
            nc.sync.dma_start(out=iden_sb, in_=iden_t.ap())
            bvec_sb = consts.tile([128, 13], f32, name="bvec_sb")
            nc.sync.dma_start(out=bvec_sb, in_=bvec_t.ap())

            qw_l = lambda kc, t: qw_sb[:, (kc * 9 + t) * 128:][:, :128]
            kw_l = lambda kc: kw_sb[:, kc * 128:][:, :128]
            vw_l = lambda kc, m: vw_sb[:, (kc * 4 + m) * 128:][:, :128]
            pw_l = lambda r, m: pw_sb[:, (r * 3 + m) * 128:][:, :128]
            vld_l = lambda r, t: vld_sb[:, (r * 9 + t) * 128:][:, :128]

            # ---- persistent augmented per-head q/k tiles (ping-pong) ----
            # rows 0:16 = per-sg head data (one contiguous DMA), rows 16:65 =
            # const identity (q) / bias table (k): S = q@k + I@bias fused.
            q2 = [[consts.tile([65, SG * 49], f16, name=f"q2_{b}_{h}")
                   for h in range(8)] for b in range(2)]
            k2 = [consts.tile([65, SG * 256], f16, name=f"k2_{h}")
                  for h in range(8)]
            it = iden_t.ap()
            bt = bias_t.ap()
            for b in range(2):
                for h in range(8):
                    nc.sync.dma_start(
                        out=q2[b][h][16:65, :],
                        in_=bass.AP(tensor=it.tensor, offset=0,
                                    ap=[[128, 49], [0, SG], [1, 49]]),
                    )
            for h in range(8):
                nc.sync.dma_start(
                    out=k2[h][16:65, :],
                    in_=bass.AP(tensor=bt.tensor, offset=h * 256,
                                ap=[[8 * 256, 49], [0, SG], [1, 256]]),
                )

            from contextlib import nullcontext

            loop_cm = tc.For_i(0, reps, 1) if reps > 1 else nullcontext()
            with loop_cm:
              for sg in range(nsg):
                e0 = sg * SG
                pp = sg % 2  # ping-pong index for q2/k2
                # ---- x for this super-group (host-padded 16x16) ----
                x_all = xp.tile([128, 3 * SG * NP], f16, name="x_sb", tag="x")
                nc.sync.dma_start(
                    out=x_all,
                    in_=bass.AP(
                        tensor=x_t.ap().tensor, offset=e0 * NP,
                        ap=[[epc * NP, 128], [128 * epc * NP, 3],
                            [NP, SG], [1, NP]],
                    ),
                )
                xr = [
                    x_all[:, kc * SG * NP : (kc + 1) * SG * NP].rearrange(
                        "p (e h w) -> p e h w", e=SG, h=HP
                    )
                    for kc in range(3)
                ]

                # ================= Q path (8 examples) =================
                ps_q = psA.tile([128, SG * 49], f32, name="ps_q", tag="ps_qk", bufs=1)
                for t in range(9):
                    dy, dx = TAPS[t]
                    for kc in range(3):
                        nc.tensor.matmul(
                            ps_q, qw_l(kc, t),
                            xr[kc][:, :, dy : dy + 13 : 2, dx : dx + 13 : 2],
                            start=(t == 0 and kc == 0),
                            stop=(t == 8 and kc == 2),
                            skip_group_check=True,
                        )
                q_sb = sbs.tile([128, SG * 49], f16, name="q_sb", tag="q")
                nc.scalar.activation(
                    out=q_sb, in_=ps_q, func=AF.Identity,
                    bias=bvec_sb[:, 12:13], scale=1.0,
                )

                # ================= K / V convs (pairs of 2 examples) ===
                k_sb = sbkv.tile([128, SG * NP], f16, name="k_sb", tag="k")
                kp = k_sb.rearrange("p (e h w) -> p e h w", e=SG, h=HP)
                if sg < 2:
                    nc.gpsimd.memset(kp[:, :, 0:16:15, :], 0.0)
                    nc.gpsimd.memset(kp[:, :, 1:15, 0:16:15], 0.0)
                v4_sb = [
                    sbkv.tile([128, SG * NP], f16, name="v4_sb", tag=f"v4_{r}")
                    for r in range(4)
                ]
                for r in range(4):
                    if sg < 2:
                        v4p = v4_sb[r].rearrange("p (e h w) -> p e h w", e=SG, h=HP)
                        nc.gpsimd.memset(v4p[:, :, 0:16:15, :], 0.0)
                        nc.gpsimd.memset(v4p[:, :, 1:15, 0:16:15], 0.0)

                for p2 in range(NPAIR):
                    es = p2 * 2
                    xin = [xr[kc][:, es : es + 2, 1:15, 1:15] for kc in range(3)]
                    ps_k = psA.tile([128, 2 * N], f32, name="ps_k", tag="ps_qk", bufs=1)
                    for kc in range(3):
                        nc.tensor.matmul(
                            ps_k, kw_l(kc), xin[kc],
                            start=(kc == 0), stop=(kc == 2),
                        )
                    nc.scalar.activation(
                        out=kp[:, es : es + 2, 1:15, 1:15], in_=ps_k,
                        func=AF.Identity, bias=bvec_sb[:, 11:12], scale=1.0,
                    )
                    for m in range(4):
                        ps_v = psA.tile([128, 2 * N], f32, name="ps_v", tag="ps_v", bufs=2)
                        for kc in range(3):
                            nc.tensor.matmul(
                                ps_v, vw_l(kc, m), xin[kc],
                                start=(kc == 0), stop=(kc == 2),
                            )
                        v4pm = v4_sb[m].rearrange("p (e h w) -> p e h w", e=SG, h=HP)
                        if m < 2:
                            nc.scalar.activation(
                                out=v4pm[:, es : es + 2, 1:15, 1:15], in_=ps_v,
                                func=AF.Identity, bias=bvec_sb[:, m : m + 1], scale=1.0,
                            )
                        else:
                            nc.vector.tensor_scalar_add(
                                out=v4pm[:, es : es + 2, 1:15, 1:15], in0=ps_v,
                                scalar1=bvec_sb[:, m : m + 1],
                            )

                # ---- per-head data rows into block-diag tiles (DMA) ----
                for h in range(8):
                    nc.sync.dma_start(
                        out=q2[pp][h][0:16, :],
                        in_=q_sb[16 * h : 16 * h + 16, :],
                    )
                    nc.sync.dma_start(
                        out=k2[h][0:16, :],
                        in_=k_sb[16 * h : 16 * h + 16, :],
                    )

                # ======== v4 -> v4T transposes (padded-flat halves) ====
                # half 0 = padded rows 0..7 (flat n' 0:128), half 1 = 8..15
                v4t = [[], []]
                for ee in range(SG):
                    ta = sbvt.tile([128, 512], f16, name="v4t_a", tag="v4t_a")
                    tb = sbvt.tile([128, 512], f16, name="v4t_b", tag="v4t_b")
                    for m in range(4):
                        for half, dst in ((0, ta), (1, tb)):
                            ps_t = psT.tile([128, 128], f16, name="ps_t", tag="pst", bufs=2)
                            nc.tensor.transpose(
                                ps_t,
                                v4_sb[m][:, ee * NP + 128 * half : ee * NP + 128 * half + 128],
                                iden_sb,
                            )
                            if (m * 2 + half) == 5:
                                nc.scalar.copy(
                                    out=dst[:, m * 128 : (m + 1) * 128], in_=ps_t
                                )
                            else:
                                nc.vector.tensor_copy(
                                    out=dst[:, m * 128 : (m + 1) * 128], in_=ps_t
                                )
                    v4t[0].append(ta)
                    v4t[1].append(tb)

                # ========= attention: S -> P -> P^T (per head2, pair) ==
                den = sbs.tile([128, 32], f32, name="den", tag="den")
                rden = sbs.tile([128, 32], f32, name="rden", tag="rden")
                at = {}
                for p in range(NPAIR):  # pair of examples 2p, 2p+1
                    a_pr = []
                    for h2 in range(4):
                        ps_s = psS.tile([128, 512], f32, name="ps_s", tag="ps_s", bufs=2)
                        for eo in range(2):
                            ee = 2 * p + eo
                            for hh in range(2):
                                h = 2 * h2 + hh
                                nc.tensor.matmul(
                                    ps_s[64 * hh : 64 * hh + 49,
                                         256 * eo : 256 * eo + 256],
                                    q2[pp][h][:, 49 * ee : 49 * ee + 49],
                                    k2[h][:, 256 * ee : 256 * ee + 256],
                                    start=True, stop=True,
                                )
                        ap = sba.tile([128, 512], f16, name="a_pr", tag="apr")
                        c = p * 8 + 2 * h2
                        for eo in range(2):
                            nc.scalar.activation(
                                out=ap[0:113, 256 * eo : 256 * eo + 256],
                                in_=ps_s[0:113, 256 * eo : 256 * eo + 256],
                                func=AF.Exp,
                                accum_out=den[0:113, c + eo : c + eo + 1],
                            )
                        a_pr.append(ap)
                    nc.vector.reciprocal(
                        out=rden[0:113, 8 * p : 8 * p + 8],
                        in_=den[0:113, 8 * p : 8 * p + 8],
                    )
                    for h2 in range(4):
                        ap = a_pr[h2]
                        c = p * 8 + 2 * h2
                        for eo in range(2):
                            nc.vector.tensor_scalar_mul(
                                out=ap[0:113, 256 * eo : 256 * eo + 256],
                                in0=ap[0:113, 256 * eo : 256 * eo + 256],
                                scalar1=rden[0:113, c + eo : c + eo + 1],
                            )
                        att = sbat.tile([128, 4 * 113], f16, name="at", tag="at")
                        for bq in range(4):  # bq = 2*eo + half
                            ps_at = psT.tile([128, 128], f16, name="ps_at", tag="pst", bufs=2)
                            nc.tensor.transpose(
                                ps_at[:, 0:113],
                                ap[:, 128 * bq : 128 * bq + 128],
                                iden_sb[:, 0:113],
                            )
                            nc.vector.tensor_copy(
                                out=att[:, 113 * bq : 113 * bq + 113],
                                in_=ps_at[:, 0:113],
                            )
                        at[(h2, p)] = att

                # ==== v_local (diag matmuls) + AV accumulate + relu ====
                z_sb = []
                for r in range(4):
                    ps_vl = psVL.tile([128, SG * 49], f32, name="ps_vl", tag="ps_vl", bufs=1)
                    v4r = v4_sb[r].rearrange("p (e h w) -> p e h w", e=SG, h=HP)
                    for ti, (dy, dx) in enumerate(TAPS):
                        nc.tensor.matmul(
                            ps_vl,
                            vld_l(r, ti),
                            v4r[:, :, dy : dy + 13 : 2, dx : dx + 13 : 2],
                            start=(ti == 0), stop=False, skip_group_check=True,
                        )
                    for ee in range(SG):
                        p, par = ee // 2, ee % 2
                        att = at[(r, p)]
                        for hh in range(2):
                            h = 2 * r + hh
                            out_sl = ps_vl[
                                64 * hh : 64 * hh + 64, ee * 49 : (ee + 1) * 49
                            ]
                            for half in range(2):
                                co = 113 * (2 * par + half) + 64 * hh
                                nc.tensor.matmul(
                                    out_sl,
                                    v4t[half][ee][:, 64 * h : 64 * h + 64],
                                    att[:, co : co + 49],
                                    start=False,
                                    stop=(ee == SG - 1 and hh == 1 and half == 1),
                                    skip_group_check=True,
                                )
                    zz = sbz.tile([128, SG * 49], f16, name="z_sb", tag=f"z{r}")
                    nc.scalar.activation(
                        out=zz, in_=ps_vl, func=AF.Relu,
                        bias=bvec_sb[:, 7 + r : 8 + r], scale=1.0,
                    )
                    z_sb.append(zz)

                # ================= projection + store =================
                yy = sby.tile([128, 3 * SG * 49], f32, name="y_sb", tag="y")
                for m in range(3):
                    ps_y = psVL.tile([128, SG * 49], f32, name="ps_y", tag="ps_vl", bufs=1)
                    for r in range(4):
                        nc.tensor.matmul(
                            ps_y, pw_l(r, m), z_sb[r],
                            start=(r == 0), stop=(r == 3),
                        )
                    nc.scalar.activation(
                        out=yy[:, m * SG * 49 : (m + 1) * SG * 49], in_=ps_y,
                        func=AF.Identity,
                        bias=bvec_sb[:, 4 + m : 5 + m], scale=1.0,
                    )
                nc.sync.dma_start(
                    out=bass.AP(
                        tensor=y_t.ap().tensor, offset=e0 * 49,
                        ap=[[epc * 49, 128], [128 * epc * 49, 3],
                            [49, SG], [1, 49]],
                    ),
                    in_=yy,
                )
    nc.compile()
    return nc


def prep_weights(inputs):
    """Host-side: fold BN/scales, build device weight layouts."""
    f = lambda a: np.asarray(a, np.float32)
    scale = KD ** -0.5

    kw2 = f(inputs["k_w"])[:, :, 0, 0] * f(inputs["k_bn_s"])[:, None]  # (128,384)
    kb2 = f(inputs["k_b"]) * f(inputs["k_bn_s"]) + f(inputs["k_bn_b"])
    qw2 = f(inputs["q_proj_w"])[:, :, 0, 0] * f(inputs["q_bn_s"])[:, None] * scale
    qb2 = (
        f(inputs["q_proj_b"]) * f(inputs["q_bn_s"]) + f(inputs["q_bn_b"])
    ) * scale + qw2 @ f(inputs["q_local_b"])
    qlw = f(inputs["q_local_w"])[:, 0].reshape(C, 9)
    vw2 = f(inputs["v_w"])[:, :, 0, 0] * f(inputs["v_bn_s"])[:, None]  # (512,384)
    vb2 = f(inputs["v_b"]) * f(inputs["v_bn_s"]) + f(inputs["v_bn_b"])
    vlw = f(inputs["vl_w"])[:, 0].reshape(DH, 9) * f(inputs["vl_bn_s"])[:, None]
    vlb = f(inputs["vl_b"]) * f(inputs["vl_bn_s"]) + f(inputs["vl_bn_b"])
    pw2 = f(inputs["p_w"])[:, :, 0, 0] * f(inputs["p_bn_s"])[:, None]  # (384,512)
    pb2 = f(inputs["p_b"]) * f(inputs["p_bn_s"]) + f(inputs["p_bn_b"])

    # 9 taps; avg-pool branch folded into the center tap (dy,dx)=(1,1)=idx 4
    qw_arr = np.zeros((128, 3, 9, 128), np.float32)
    for kc in range(3):
        cs = slice(128 * kc, 128 * kc + 128)
        for t in range(9):
            lw = qlw[cs, t] + (1.0 if t == 4 else 0.0)
            qw_arr[:, kc, t, :] = qw2[:, cs].T * lw[:, None]
    kw_arr = np.zeros((128, 3, 128), np.float32)
    for kc in range(3):
        kw_arr[:, kc, :] = kw2[:, 128 * kc : 128 * kc + 128].T
    vw_arr = np.zeros((128, 3, 4, 128), np.float32)
    for kc in range(3):
        for m in range(4):
            vw_arr[:, kc, m, :] = vw2[128 * m : 128 * m + 128, 128 * kc : 128 * kc + 128].T
    pw_arr = np.zeros((128, 4, 3, 128), np.float32)
    for r in range(4):
        for m in range(3):
            pw_arr[:, r, m, :] = pw2[128 * m : 128 * m + 128, 128 * r : 128 * r + 128].T
    vld_arr = np.zeros((128, 4, 9, 128), np.float32)
    ii = np.arange(128)
    for r in range(4):
        for t in range(9):
            vld_arr[ii, r, t, ii] = vlw[128 * r : 128 * r + 128, t]

    # padded bias: (49, 8, 256) in 16x16 flat index; pad cols get -60000
    bt = f(inputs["bias_tab"]).transpose(1, 0, 2).reshape(49, 8, 14, 14)
    bias_arr = np.full((49, 8, HP, WP), -60000.0, np.float32)
    bias_arr[:, :, 1:15, 1:15] = bt
    bias_arr = bias_arr.reshape(49, 8, 256)

    bvec = np.zeros((128, 13), np.float32)
    for m in range(4):
        bvec[:, m] = vb2[128 * m : 128 * m + 128]
    for m in range(3):
        bvec[:, 4 + m] = pb2[128 * m : 128 * m + 128]
    for r in range(4):
        bvec[:, 7 + r] = vlb[128 * r : 128 * r + 128]
    bvec[:, 11] = kb2
    bvec[:, 12] = qb2

    return {
        "qw": qw_arr.astype(np.float16),
        "kw": kw_arr.astype(np.float16),
        "vw": vw_arr.astype(np.float16),
        "pw": pw_arr.astype(np.float16),
        "vld": vld_arr.astype(np.float16),
        "bias": bias_arr.astype(np.float16),
        "iden": np.eye(128, dtype=np.float16),
        "bvec": bvec,
    }


def prep_x_core(x, c, epc=EPC):
    """x (B, C, H, W) -> per-core (3, 128, epc, 256) fp16 with zero halo."""
    xc = np.asarray(x, np.float32)[c * epc : (c + 1) * epc]  # (epc, C, 14, 14)
    xp = np.zeros((epc, C, HP, WP), np.float32)
    xp[:, :, 1:15, 1:15] = xc
    xp = xp.reshape(epc, C, NP).transpose(1, 0, 2).reshape(3, 128, epc, NP)
    return xp.astype(np.float16)


def unpack_y(y, epc=EPC):
    """(3, 128, epc, 49) fp32 -> (epc, 384, 7, 7)."""
    return (
        np.asarray(y, np.float32)
        .reshape(OUT, epc, 49)
        .transpose(1, 0, 2)
        .reshape(epc, OUT, 7, 7)
    )


_CACHE = {}


def kernel(**inputs) -> np.ndarray:
    from concourse import bass_utils

    if "nc" not in _CACHE:
        _CACHE["nc"] = build_bass()
    nc = _CACHE["nc"]

    wmaps = prep_weights(inputs)
    in_maps = []
    for c in range(NCORES):
        m = dict(wmaps)
        m["x"] = prep_x_core(inputs["x"], c)
        in_maps.append(m)

    res = bass_utils.run_bass_kernel_spmd(nc, in_maps, core_ids=list(range(NCORES)))
    outs = [unpack_y(r["y"]) for r in res.results]
    return np.concatenate(outs, axis=0)


if __name__ == "__main__":
    print("building bass program...")
    nc = build_bass()
    print("build OK")
